# revision 28
# baseline (speedup 1.0000x reference)
"""Trainium2 Bass kernel for nn_Aggre_user (GNN message-passing aggregation).

Reference computation (per batch node, over its variable-length edge list):
    f      = relu(ln1(cat(user_emb, rating_emb)))            per edge
    h      = relu(att2(relu(att1(cat(f, item_emb[node])))))  per edge
    a      = att3(h)                                         per edge logit
    mu     = segment_softmax(a)
    z      = relu(ln2(segment_sum(f * mu)))                  per node
    out    = relu(ln3(cat(item_emb[node], z)))               per node

Sharding: nodes (B=8192) split contiguously across 8 cores (1024 each);
segment_ids are sorted, so each node's edges land wholly on one core.
No collectives needed.

V2 design (vs. the earlier edge-major baseline):
  - The user table is folded through ln1 on the HOST: ctab[u] = bf16(W1u @ u)
    stored as 256B rows [c_u(64) | zeros(64)].  The per-edge gather runs the
    ANT dma_gather ucode in TRANSPOSE mode, so c_u arrives FEATURE-MAJOR:
    no ln1-u matmul, no PE transposes of u, no dtype cast.
  - Per-call num_idxs is capped at the 8-core max real-slot count of the
    window (128-aligned for transpose mode) so all-pad tails cost nothing.
    (Shipping pads as -1 for the ucode's per-core self-trim measured as a
    hard crash in transpose mode -- the rx spray path desyncs -- so pads
    are plain index 0 and their contributions die in the segment one-hot.)
  - Tile pairs [cA; cB] are assembled for the 128-row block-diagonal
    pipeline with 4 SBUF->SBUF DMA copies per quad (DMA engines are
    otherwise idle; the bottom halves need a partition shift that DVE
    lanes cannot do), then ONE full-width [128,512] DVE add folds them
    onto the rating PSUM. Half-partition [64,x] DVE ops run at half rate
    and also stall the gather ucode via the shared SBUF port -- avoid.
  - att1's per-node one-hot (node-major oh1) is built WITHOUT PE transposes:
    a 2-row broadcast matmul (lhsT = half-selector, rhs = host-shipped
    pairwise seg-local ids) replicates seg ids down the partitions, then one
    DVE is_equal against a per-partition iota gives oh1 directly.
  - Item embeddings: host packs FOUR items per 1024B row -> the item table
    has 25000 rows = ONE int16 window; four 256-idx transpose-mode calls
    (1024B elems generate 4 rx descriptors per index, so a single 1024-idx
    call would overflow the 32KB SWDGE descriptor ring) fetch all nodes'
    items in one pass; chunk masks select the right 64-row slot.
  - rating table folded into ln1 via host-built 5-hot and M_r = rating_table
    @ ln1_w[:,64:].T streamed as extra contraction rows (10-row lhsT).
  - segment softmax: no max-subtraction needed (logits O(1)); no explicit mu:
    z = (sum_e ex*f) / max(sum_e ex, 1e-9); both sums via one matmul with an
    ex-scaled one-hot and a ones column (per shard-cell partials, summed).
"""

import math
import numpy as np
import ml_dtypes

import concourse.bass as bass
import concourse.mybir as mybir
import concourse.tile as tile
from concourse import bacc
from concourse.bass_utils import run_bass_kernel_spmd

BF16 = ml_dtypes.bfloat16
AF = mybir.ActivationFunctionType
N_CORES = 8
W = 64        # nodes per segment group
SH = 25000    # ids per gather shard
NSH = 4       # shards
CALL_T = 4    # tiles per user-gather call
NEG_TRIM = False  # -1 pads crash the transpose gather ucode (tx/rx desync)
QUEUES = 1    # SWDGE queues; queue q runs on Q7 core pair (2q, 2q+1)
NEG_ALIGN = 128  # per-core trim alignment


def pack_item_table(t):
    """[N, 64] f32 -> quad-packed bf16 rows [i0|0|i1|0|i2|0|i3|0] (1024B).

    25000 rows fit ONE int16 window, so the whole item gather is a single
    transpose-mode pass; per-node chunk masks select the right 64-row slot.
    """
    t = np.asarray(t, np.float32)
    N = t.shape[0]
    R = (N + 3) // 4
    out = np.zeros((R, 512), np.float32)
    for k in range(4):
        rows = t[k::4]
        out[: len(rows), 128 * k: 128 * k + 64] = rows
    return out.astype(BF16), R


def wrap16(idx_i16):
    """flat int16 index list (len % 16 == 0) -> [128, len//16] wrapped+replicated."""
    a = idx_i16.reshape(-1, 16).T  # [16, S]
    return np.tile(a, (8, 1)).copy()


# ----------------------------------------------------------------------------
# host-side preprocessing: shard + pad + relayout (pure index manipulation)
# ----------------------------------------------------------------------------

def host_prep(inputs, n_cores=N_CORES, call_t=CALL_T):
    user_idx = np.asarray(inputs["user_idx"]).astype(np.int64)
    rating_idx = np.asarray(inputs["rating_idx"]).astype(np.int64)
    item_idx = np.asarray(inputs["item_idx"]).astype(np.int64)
    seg = np.asarray(inputs["segment_ids"]).astype(np.int64)

    B = item_idx.shape[0]
    B_loc = B // n_cores
    assert B_loc % 128 == 0
    NG = B_loc // W
    n_groups = B // W

    f32 = np.float32
    ln1_w = np.asarray(inputs["ln1_w"], f32)

    # host-folded user table: ctab[u] = [W1u @ u | 0] in bf16 (256B rows)
    utab_f32 = np.ascontiguousarray(np.asarray(inputs["user_table"], f32))
    c_all = utab_f32 @ ln1_w[:, :64].T            # [N_u, 64] f32
    N_u = c_all.shape[0]
    ctab = np.zeros((N_u, 128), np.float32)
    ctab[:, 0:64] = c_all
    ctab = ctab.astype(BF16)

    itab, n_irows = pack_item_table(inputs["item_table"])
    n_ush = (N_u + SH - 1) // SH
    u_shard = user_idx // SH        # shard per edge

    bounds = np.searchsorted(seg, np.arange(n_groups + 1) * W)

    # per-(group, shard) cell counts -> per-shard cell capacity C_q
    cellcnt = np.zeros((n_groups, NSH), np.int64)
    for g in range(n_groups):
        lo, hi = bounds[g], bounds[g + 1]
        for q in range(NSH):
            cellcnt[g, q] = int((u_shard[lo:hi] == q).sum())
    # tiles per shard block must divide into CALL_T-tile gather calls:
    # NG * C_q / 128 % CALL_T == 0  <=>  C_q % (128 * CALL_T / NG) == 0
    align = max(128, 128 * call_t // NG)
    Cq = [0 if q >= n_ush else
          int(math.ceil(max(1, int(cellcnt[:, q].max())) / align) * align)
          for q in range(NSH)]
    E_grp = sum(Cq)
    T = NG * E_grp // 128
    assert T % 8 == 0
    E_pad = NG * E_grp
    blk_tiles = [NG * c // 128 for c in Cq]   # tiles per shard block
    group_of_tile = []
    for q in range(NSH):
        for g in range(NG):
            group_of_tile += [g] * (Cq[q] // 128)
    shard_of_tile = []
    for q in range(NSH):
        shard_of_tile += [q] * blk_tiles[q]

    per_core = []
    blk0 = np.concatenate([[0], np.cumsum(blk_tiles)]) * 128  # slot offsets

    # Within a cell, slot order is free. Place each cell's pad slots as one
    # run ending at the last 512-boundary inside the cell: every gather-call
    # window the run touches has it as a suffix. Suffix pads are shipped as
    # index -1: the ucode trims trailing negatives PER CORE, so their
    # descriptor generation cost vanishes. The static per-call num_idxs cap
    # (8-core max, rounded to 128 for transpose mode) also skips their
    # index-load cost where all 8 cores are padded.
    win = 128 * call_t
    n_calls = T // call_t
    all_lastreal = []
    for k in range(n_cores):
        ugl = np.zeros(E_pad, np.int64)      # shard-local padded positions
        ridx = np.full(E_pad, -1, np.int64)
        sloc = np.full(E_pad, -1.0, np.float64)
        is_pad = np.zeros(E_pad, bool)
        for gl in range(NG):
            g = NG * k + gl
            lo, hi = bounds[g], bounds[g + 1]
            esl = np.arange(lo, hi)
            shards_here = u_shard[lo:hi]
            for q in range(NSH):
                if Cq[q] == 0:
                    assert not (shards_here == q).any()
                    continue
                mine = esl[shards_here == q]
                c = len(mine)
                s = int(blk0[q]) + gl * Cq[q]
                assert c <= Cq[q]
                p = Cq[q] - c
                bnd = ((s + Cq[q]) // win) * win
                if bnd - p < s or bnd <= s:
                    bnd = s + Cq[q]  # no usable boundary: pads at cell end
                pos = np.concatenate([np.arange(s, bnd - p),
                                      np.arange(bnd, s + Cq[q])])
                assert len(pos) == c
                ugl[pos] = user_idx[mine] - SH * q
                is_pad[bnd - p:bnd] = True
                ridx[pos] = rating_idx[mine]
                sloc[pos] = seg[mine] - W * g
        assert (ugl >= 0).all() and (ugl < SH).all()
        # mark window-suffix pads as -1 (ucode self-trim); others stay 0.
        # The trimmed per-core count must stay a multiple of 16: the
        # transpose rx path sprays full 16-lane groups unconditionally, so a
        # ragged trim desyncs tx/rx descriptor counts (hang). Trim only from
        # the 16-aligned boundary above the last real slot.
        lastreal = []
        for c2 in range(n_calls):
            wsl = slice(c2 * win, (c2 + 1) * win)
            nz = np.nonzero(~is_pad[wsl])[0]
            last = int(nz[-1]) + 1 if len(nz) else 0
            lastreal.append(last)
            if NEG_TRIM:
                last16 = -(-last // NEG_ALIGN) * NEG_ALIGN
                ugl[c2 * win + last16:(c2 + 1) * win] = -1
        all_lastreal.append(lastreal)
        uw = wrap16(ugl.astype(np.int16))    # [128, E_pad//16]
        segl = sloc.reshape(T, 128).T.astype(BF16).copy()
        # pairwise seg-local ids for the oh1 broadcast matmul:
        # col 128*p + e -> (row0: tile 2p, row1: tile 2p+1)
        sl2 = sloc.reshape(T, 128)
        seglAB = np.stack([sl2[0::2].reshape(-1), sl2[1::2].reshape(-1)])
        rt = ridx.reshape(T, 128)
        P = T // 2
        oh5 = np.zeros((10, P, 128), np.float32)
        for r in range(5):
            oh5[r] = (rt[0::2] == r)
            oh5[5 + r] = (rt[1::2] == r)
        oh5p = oh5.reshape(10, P * 128).astype(BF16)
        # item gather: quad-packed single window, node order
        nodes = slice(B_loc * k, B_loc * (k + 1))
        it = item_idx[nodes]
        iw = wrap16((it // 4).astype(np.int16))
        masks = []
        for kk in range(4):
            m = np.tile((it % 4 == kk).astype(np.float32), (128, 1))
            masks.append(m.astype(BF16))
        per_core.append(dict(
            uw=uw, segl=segl, seglAB=seglAB.astype(BF16), oh5p=oh5p,
            iw=iw, im=np.stack(masks, axis=1),  # [128, 4, B_loc]
        ))

    call_caps = [
        min(win, -(-max(all_lastreal[k][c] for k in range(n_cores)) // 128) * 128)
        for c in range(n_calls)
    ]

    # weights (tiny; fold rating table into ln1 on host)
    att1_w = np.asarray(inputs["att1_w"], f32)
    att2_w = np.asarray(inputs["att2_w"], f32)
    att3_w = np.asarray(inputs["att3_w"], f32)
    ln2_w = np.asarray(inputs["ln2_w"], f32)
    ln3_w = np.asarray(inputs["ln3_w"], f32)
    rating_table = np.asarray(inputs["rating_table"], f32)

    def bd(a):
        K, M = a.shape
        o = np.zeros((2 * K, 2 * M), f32)
        o[:K, :M] = a
        o[K:, M:] = a
        return o.astype(BF16)

    MrT = rating_table @ ln1_w[:, 64:].T
    w3 = att3_w[0]
    w3p = np.zeros((128, 2), f32)
    w3p[:64, 0] = w3
    w3p[64:, 1] = w3

    sel2 = np.zeros((2, 128), f32)
    sel2[0, 0:64] = 1.0
    sel2[1, 64:128] = 1.0
    iota64p = np.tile(np.arange(W, dtype=f32), 2)[:, None]  # [128,1] p%64

    shared = dict(
        bd_mr=bd(MrT),
        bd_a1f=bd(att1_w[:, :64].T),
        a1it=att1_w[:, 64:].T.astype(BF16),
        bd_a2=bd(att2_w.T), w3p=w3p.astype(BF16),
        w2t=ln2_w.T.astype(BF16),
        w3it=ln3_w[:, :64].T.astype(BF16), w3zt=ln3_w[:, 64:].T.astype(BF16),
        b1p=np.tile(np.asarray(inputs["ln1_b"], f32), 2)[:, None],
        ba1p=np.tile(np.asarray(inputs["att1_b"], f32), 2)[:, None],
        ba2p=np.tile(np.asarray(inputs["att2_b"], f32), 2)[:, None],
        b2=np.asarray(inputs["ln2_b"], f32)[:, None],
        b3f=np.asarray(inputs["ln3_b"], f32)[:, None],
        iota64=np.tile(np.arange(W, dtype=f32), (128, 1)).astype(BF16),
        id_bf=np.eye(128, dtype=f32).astype(BF16),
        id_f32=np.eye(128, dtype=f32),
        sel2=sel2.astype(BF16), iota64p=iota64p.astype(f32),
        ctab=ctab, itab=itab,
    )
    meta = dict(B=B, B_loc=B_loc, NG=NG, T=T, E_grp=E_grp, Cq=tuple(Cq),
                blk_tiles=tuple(blk_tiles), group_of_tile=tuple(group_of_tile),
                shard_of_tile=tuple(shard_of_tile),
                call_caps=tuple(call_caps),
                n_cores=n_cores, UR=ctab.shape[0], IR=itab.shape[0],
                n_ush=n_ush, call_t=call_t, ver=2)
    return per_core, shared, meta


# ----------------------------------------------------------------------------
# bass program builder
# ----------------------------------------------------------------------------

def build_nc_real(meta):
    NG, T = meta["NG"], meta["T"]
    B_loc = meta["B_loc"]
    Cq, blk_tiles = meta["Cq"], meta["blk_tiles"]
    got = meta["group_of_tile"]
    sot = meta["shard_of_tile"]
    CT = meta["call_t"]
    assert T % CT == 0
    n_calls = T // CT

    nq = meta.get("queues", 1)
    nc = bacc.Bacc("TRN2", target_bir_lowering=False, debug=False,
                   enable_asserts=False, num_devices=meta["n_cores"],
                   dynamic_dma_scratch_size=32768, num_swdge_queues=nq)
    f32, bf16 = mybir.dt.float32, mybir.dt.bfloat16
    i16, i32 = mybir.dt.int16, mybir.dt.int32

    def din(name, shape, dtype):
        return nc.dram_tensor(name, shape, dtype, kind="ExternalInput").ap()

    ctab = din("ctab", [meta["UR"], 128], bf16)
    itab = din("itab", [meta["IR"], 512], bf16)
    uw = din("uw", [128, T * 8], i16)
    segl = din("segl", [128, T], bf16)
    seglAB = din("seglAB", [2, T * 64], bf16)
    oh5p = din("oh5p", [10, 64 * T], bf16)
    iw = din("iw", [128, B_loc // 16], i16)
    im = din("im", [128, 4, B_loc], bf16)
    iota64 = din("iota64", [128, W], bf16)
    id_bf = din("id_bf", [128, 128], bf16)
    id_f32 = din("id_f32", [128, 128], f32)
    sel2 = din("sel2", [2, 128], bf16)
    iota64p = din("iota64p", [128, 1], f32)
    bd_mr = din("bd_mr", [10, 128], bf16)
    bd_a1f = din("bd_a1f", [128, 128], bf16)
    a1it = din("a1it", [64, 64], bf16)
    bd_a2 = din("bd_a2", [128, 128], bf16)
    w3p = din("w3p", [128, 2], bf16)
    w2t = din("w2t", [64, 64], bf16)
    w3it = din("w3it", [64, 64], bf16)
    w3zt = din("w3zt", [64, 64], bf16)
    b1p = din("b1p", [128, 1], f32)
    ba1p = din("ba1p", [128, 1], f32)
    ba2p = din("ba2p", [128, 1], f32)
    b2 = din("b2", [64, 1], f32)
    b3f = din("b3f", [64, 1], f32)
    out = nc.dram_tensor("out", [B_loc, 64], f32, kind="ExternalOutput").ap()

    with tile.TileContext(nc) as tc:
        with (
            tc.tile_pool(name="const", bufs=1) as cpool,
            tc.tile_pool(name="core", bufs=1) as corep,
            tc.tile_pool(name="ug", bufs=12) as ugp,
            tc.tile_pool(name="stk", bufs=3) as stkp,
            tc.tile_pool(name="sab", bufs=3) as sabp,
            tc.tile_pool(name="qsb", bufs=4) as qsb,
            tc.tile_pool(name="post", bufs=2) as postp,
            tc.tile_pool(name="pmm", bufs=3, space="PSUM") as pmm,
            tc.tile_pool(name="ptr", bufs=2, space="PSUM") as ptr,
            tc.tile_pool(name="pa", bufs=1, space="PSUM") as pa,
            tc.tile_pool(name="pg", bufs=1, space="PSUM") as pg,
            tc.tile_pool(name="pgp", bufs=1, space="PSUM") as pgp,
        ):
            def load(pool, ap, tag):
                t = pool.tile(list(ap.shape), ap.dtype, tag=tag, name=tag)
                nc.sync.dma_start(out=t[:], in_=ap)
                return t

            c_id_bf = load(cpool, id_bf, "id_bf")
            c_id_f32 = load(cpool, id_f32, "id_f32")
            c_iota = load(cpool, iota64, "iota")
            c_sel2 = load(cpool, sel2, "sel2")
            c_iota64p = load(cpool, iota64p, "iota64p")
            c_bd_mr = load(cpool, bd_mr, "bd_mr")
            c_bd_a1f = load(cpool, bd_a1f, "bd_a1f")
            c_a1it = load(cpool, a1it, "a1it")
            c_bd_a2 = load(cpool, bd_a2, "bd_a2")
            c_w3p = load(cpool, w3p, "w3p")
            c_w2t = load(cpool, w2t, "w2t")
            c_w3it = load(cpool, w3it, "w3it")
            c_w3zt = load(cpool, w3zt, "w3zt")
            c_b1p = load(cpool, b1p, "b1p")
            c_ba1p = load(cpool, ba1p, "ba1p")
            c_ba2p = load(cpool, ba2p, "ba2p")
            c_b2 = load(cpool, b2, "b2")
            c_b3f = load(cpool, b3f, "b3f")
            c_segl = load(corep, segl, "segl")
            c_uw = load(corep, uw, "uw")
            c_oh5p = load(corep, oh5p, "oh5p")
            c_iw = load(corep, iw, "iw")
            c_im = load(corep, im, "im")

            # Zero the user-gather ring once: slots past a call's cap are
            # never written by the gather, so their SBUF content must be
            # finite (contributions are killed by the segment one-hot, but
            # Inf/NaN garbage would poison 0*x products downstream).
            for _i in range(12):
                zt = ugp.tile([128, CT * 128], bf16, tag="u_g",
                              name=f"ug_init{_i}")
                nc.vector.memset(zt[:], 0.0)

            dbg = meta.get("dbg", "")

            for _rep in range(meta.get("repeat", 1)):
                # ---- item embeddings: one quad-packed transpose pass ----
                # 1024B rows -> 4 rx descriptors per index; chunk calls to
                # 256 idxs so each call's descriptors fit the 32KB SWDGE ring.
                # Layout [128, call, chunk, idx]: each call's region is
                # contiguous; chunk k is read back with a strided view.
                ICH = 256
                NIC = B_loc // ICH
                gq = corep.tile([128, NIC, 4, ICH], bf16, tag="itg", name="itg")
                if "no_item" in dbg:
                    nc.vector.memset(gq[:], 0.0)
                else:
                    for c in range(NIC):
                        nc.gpsimd.dma_gather(
                            out_ap=gq[:, c, :, :],
                            in_ap=itab[:, :],
                            idxs_ap=c_iw[:, c * ICH // 16:(c + 1) * ICH // 16],
                            num_idxs=ICH, num_idxs_reg=ICH,
                            elem_size=512, transpose=True,
                            queue_num=c % nq)

                def gq_chunk(k):
                    return gq[:, :, k, :]                # [128, NIC, ICH]

                def imv(k):
                    return c_im[:, k, :].rearrange("p (c n) -> p c n", c=NIC)

                s0 = corep.tile([128, B_loc], bf16, tag="s0", name="s0")
                s1 = corep.tile([128, B_loc], bf16, tag="s1", name="s1")
                sv = [s.rearrange("p (c n) -> p c n", c=NIC)
                      for s in (s0[:], s1[:])]
                nc.vector.tensor_tensor(out=sv[0], in0=gq_chunk(0),
                                        in1=imv(0), op=mybir.AluOpType.mult)
                nc.vector.tensor_tensor(out=sv[1], in0=gq_chunk(1),
                                        in1=imv(1), op=mybir.AluOpType.mult)
                s2 = corep.tile([128, B_loc], bf16, tag="s2", name="s2")
                s3 = corep.tile([128, B_loc], bf16, tag="s3", name="s3")
                sv2 = [s.rearrange("p (c n) -> p c n", c=NIC)
                       for s in (s2[:], s3[:])]
                nc.vector.tensor_tensor(out=sv2[0], in0=gq_chunk(2),
                                        in1=imv(2), op=mybir.AluOpType.mult)
                nc.vector.tensor_tensor(out=sv2[1], in0=gq_chunk(3),
                                        in1=imv(3), op=mybir.AluOpType.mult)
                nc.vector.tensor_tensor(out=s0[:], in0=s0[:], in1=s1[:],
                                        op=mybir.AluOpType.add)
                nc.vector.tensor_tensor(out=s2[:], in0=s2[:], in1=s3[:],
                                        op=mybir.AluOpType.add)
                itemT = corep.tile([128, B_loc], bf16, tag="itemT", name="itemT")
                nc.vector.tensor_tensor(out=itemT[:], in0=s0[:], in1=s2[:],
                                        op=mybir.AluOpType.add)

                def item_fm_slice(g):
                    return itemT[0:64, W * g:W * g + W]

                # ---- per-group c1 block-diag lhsT ----
                bd_c1 = corep.tile([128, NG, 128], bf16, tag="bd_c1", name="bd_c1")
                nc.gpsimd.memset(bd_c1[:], 0)
                for g in range(NG):
                    src = item_fm_slice(g)
                    ps = pgp.tile([128, 128], f32, tag="gp", name=f"c1ps{g}")
                    nc.tensor.matmul(ps[0:64, 0:64], lhsT=src, rhs=c_a1it[:],
                                     start=True, stop=True, skip_group_check=True)
                    nc.tensor.matmul(ps[64:128, 64:128], lhsT=src, rhs=c_a1it[:],
                                     start=True, stop=True, skip_group_check=True)
                    nc.vector.tensor_copy(out=bd_c1[0:64, g, 0:64],
                                          in_=ps[0:64, 0:64])
                    nc.vector.tensor_copy(out=bd_c1[64:128, g, 64:128],
                                          in_=ps[64:128, 64:128])

                # per-group accumulated G (f32, SBUF)
                G_all = corep.tile([65, NG, W], f32, tag="G_all", name="G_all")

                def group_post(g):
                    G_sb = G_all[:, g, :]
                    Gt = pgp.tile([64, 65], f32, tag="gp", name=f"Gt{g}")
                    nc.tensor.transpose(out=Gt[:], in_=G_sb,
                                        identity=c_id_f32[0:65, 0:65])
                    den = postp.tile([64, 1], f32, tag="den", name=f"den{g}")
                    nc.vector.tensor_scalar_max(out=den[:], in0=Gt[:, 64:65],
                                                scalar1=1e-9)
                    rec = postp.tile([64, 1], f32, tag="rec", name=f"rec{g}")
                    nc.vector.reciprocal(out=rec[:], in_=den[:])
                    z_nm = postp.tile([64, W], bf16, tag="z_nm", name=f"znm{g}")
                    nc.vector.tensor_scalar_mul(out=z_nm[:], in0=Gt[:, 0:64],
                                                scalar1=rec[:, 0:1])
                    zf_ps = pgp.tile([64, 64], bf16, tag="gp", name=f"zf{g}")
                    nc.tensor.transpose(out=zf_ps[:], in_=z_nm[:],
                                        identity=c_id_bf[0:64, 0:64])
                    z_fm = postp.tile([64, 64], bf16, tag="z_fm", name=f"zfm{g}")
                    nc.scalar.activation(out=z_fm[:], in_=zf_ps[:],
                                         func=AF.Copy)
                    z2_ps = pgp.tile([64, 64], f32, tag="gp", name=f"z2ps{g}")
                    nc.tensor.matmul(z2_ps[:], lhsT=c_w2t[:], rhs=z_fm[:],
                                     start=True, stop=True, skip_group_check=True)
                    z2 = postp.tile([64, 64], bf16, tag="z2", name=f"z2{g}")
                    nc.scalar.activation(out=z2[:], in_=z2_ps[:], func=AF.Relu,
                                         bias=c_b2[:])
                    o_ps = pgp.tile([64, 64], f32, tag="gp", name=f"ops{g}")
                    nc.tensor.matmul(o_ps[:], lhsT=c_w3it[:], rhs=item_fm_slice(g),
                                     start=True, stop=False, skip_group_check=True)
                    nc.tensor.matmul(o_ps[:], lhsT=c_w3zt[:], rhs=z2[:],
                                     start=False, stop=True, skip_group_check=True)
                    o_fm = postp.tile([64, 64], f32, tag="o_fm", name=f"ofm{g}")
                    nc.scalar.activation(out=o_fm[:], in_=o_ps[:], func=AF.Relu,
                                         bias=c_b3f[:])
                    otr = pgp.tile([64, 64], f32, tag="gp", name=f"otr{g}")
                    nc.tensor.transpose(out=otr[:], in_=o_fm[:],
                                        identity=c_id_f32[0:64, 0:64])
                    o_sb = postp.tile([64, 64], f32, tag="o_sb", name=f"osb{g}")
                    nc.scalar.activation(out=o_sb[:], in_=otr[:],
                                         func=AF.Copy)
                    nc.sync.dma_start(out=out[W * g:W * g + W, :], in_=o_sb[:])

                # ---- main edge pipeline over gather calls / quads ----
                blk_first = [sum(blk_tiles[:q]) for q in range(NSH)]
                last_q = max(q for q in range(NSH) if blk_tiles[q] > 0)
                first_q = min(q for q in range(NSH) if blk_tiles[q] > 0)
                G_cell = {}

                u_call = {}
                caps = meta["call_caps"]

                def ensure_call(call):
                    if call in u_call:
                        return
                    t0 = call * CT
                    q_shard = sot[t0]
                    base = SH * q_shard
                    rows = min(SH, meta["UR"] - base)
                    cap = caps[call]
                    u_g = ugp.tile([128, CT * 128], bf16, tag="u_g",
                                   name=f"ug{call}")
                    if "no_ug" in dbg:
                        nc.vector.memset(u_g[:], 0.0)
                    elif cap > 0:
                        nc.gpsimd.dma_gather(
                            out_ap=u_g[:].rearrange("p (o n) -> p o n",
                                                    o=1)[:, :, 0:cap],
                            in_ap=ctab[base:base + rows, :],
                            idxs_ap=c_uw[:, t0 * 8:t0 * 8 + cap // 16],
                            num_idxs=cap, num_idxs_reg=cap,
                            elem_size=128, transpose=True,
                            queue_num=call % nq)
                    u_call[call] = u_g

                for quad in range(T // 8):
                    if True:
                        qt0 = quad * 8
                        cA = qt0 // CT
                        cB = cA + 1
                        for c in (cA, cB):
                            ensure_call(c)
                        if meta.get("only_gather"):
                            continue

                        # --- bottom halves of the tile pairs need a
                        # partition shift: SBUF->SBUF DMA from gather ring
                        # rows 0:64 (odd tiles) into stk rows 64:128 ---
                        stk = stkp.tile([128, 512], bf16, tag="stk")
                        if "no_stk" in dbg:
                            nc.vector.memset(stk[:], 0.0)
                        else:
                            for ci, cc in enumerate((cA, cB)):
                                nc.sync.dma_start(
                                    out=stk[0:64, 256 * ci:256 * ci + 256]
                                    .rearrange("p (b n) -> p b n", b=2),
                                    in_=u_call[cc][0:64, :]
                                    .rearrange("p (b n) -> p b n", b=4)[:, 0::2, :])
                                nc.sync.dma_start(
                                    out=stk[64:128, 256 * ci:256 * ci + 256]
                                    .rearrange("p (b n) -> p b n", b=2),
                                    in_=u_call[cc][0:64, :]
                                    .rearrange("p (b n) -> p b n", b=4)[:, 1::2, :])

                        # --- ln1: rating part on PE, user part added in ---
                        f_ps = pmm.tile([128, 512], f32, tag="mm")
                        nc.tensor.matmul(
                            f_ps[:], lhsT=c_bd_mr[:],
                            rhs=c_oh5p[:, 64 * qt0:64 * qt0 + 512],
                            start=True, stop=True, skip_group_check=True)
                        f_pre = qsb.tile([128, 512], bf16, tag="fpre")
                        nc.vector.tensor_tensor(
                            out=f_pre[:],
                            in0=f_ps[:],
                            in1=stk[:],
                            op=mybir.AluOpType.add)
                        f_sb = qsb.tile([128, 512], bf16, tag="f")
                        nc.scalar.activation(out=f_sb[:], in_=f_pre[:],
                                             func=AF.Relu, bias=c_b1p[:])

                        # --- segment one-hots ---
                        oh2 = qsb.tile([128, 8, W], bf16, tag="oh2")
                        nc.vector.tensor_tensor(
                            out=oh2[:],
                            in0=c_segl[:, qt0:qt0 + 8].to_broadcast([128, 8, W]),
                            in1=c_iota[:].rearrange("p (o n) -> p o n", o=1)
                                .to_broadcast([128, 8, W]),
                            op=mybir.AluOpType.is_equal,
                        )
                        # oh1 (node-major) via 2-row broadcast matmul +
                        # per-partition iota compare (no PE transposes)
                        sAB = sabp.tile([2, 512], bf16, tag="sab")
                        nc.sync.dma_start(
                            out=sAB[:], in_=seglAB[:, 64 * qt0:64 * qt0 + 512])
                        bc_ps = pmm.tile([128, 512], f32, tag="mm")
                        nc.tensor.matmul(
                            bc_ps[:], lhsT=c_sel2[:],
                            rhs=sAB[:],
                            start=True, stop=True, skip_group_check=True)
                        oh1 = qsb.tile([128, 512], bf16, tag="oh1")
                        nc.vector.tensor_scalar(
                            out=oh1[:], in0=bc_ps[:], scalar1=c_iota64p[:],
                            scalar2=None, op0=mybir.AluOpType.is_equal)

                        # --- att1 (f-part batched; c1-part per group run) ---
                        h1_ps = pmm.tile([128, 512], f32, tag="mm")
                        nc.tensor.matmul(h1_ps[:], lhsT=c_bd_a1f[:],
                                         rhs=f_sb[:], start=True,
                                         stop=False, skip_group_check=True)
                        p = 0
                        while p < 4:
                            g = got[qt0 + 2 * p]
                            p2 = p
                            while p2 < 4 and got[qt0 + 2 * p2] == g:
                                p2 += 1
                            cs = slice(128 * p, 128 * p2)
                            nc.tensor.matmul(h1_ps[:, cs], lhsT=bd_c1[:, g, :],
                                             rhs=oh1[:, cs], start=False,
                                             stop=True, skip_group_check=True)
                            p = p2
                        h1 = qsb.tile([128, 512], bf16, tag="h1")
                        nc.scalar.activation(out=h1[:], in_=h1_ps[:], func=AF.Relu,
                                             bias=c_ba1p[:])

                        # --- att2 ---
                        h2_ps = pmm.tile([128, 512], f32, tag="mm")
                        nc.tensor.matmul(h2_ps[:], lhsT=c_bd_a2[:],
                                         rhs=h1[:], start=True, stop=True,
                                         skip_group_check=True)
                        h2 = qsb.tile([128, 512], bf16, tag="h2")
                        nc.scalar.activation(out=h2[:], in_=h2_ps[:],
                                             func=AF.Relu, bias=c_ba2p[:])

                        # --- att3 logits + exp ---
                        a_ps = pa.tile([2, 512], f32, tag="a")
                        nc.tensor.matmul(a_ps[:], lhsT=c_w3p[:], rhs=h2[:],
                                         start=True, stop=True, skip_group_check=True)
                        ex = qsb.tile([2, 512], bf16, tag="ex")
                        nc.scalar.activation(out=ex[:], in_=a_ps[:], func=AF.Exp)

                        ex_ps = ptr.tile([128, 8], bf16, tag="tr")
                        for p in range(4):
                            nc.tensor.transpose(
                                out=ex_ps[:, 2 * p:2 * p + 2],
                                in_=ex[:, 128 * p:128 * (p + 1)],
                                identity=c_id_bf[0:2, 0:2],
                            )
                        ex_em = qsb.tile([128, 8], bf16, tag="ex_em")
                        nc.scalar.activation(out=ex_em[:], in_=ex_ps[:],
                                             func=AF.Copy)

                        # --- f to edge-major, ex folded in: fT = [f*ex; ex]
                        # so G's rhs is oh2 directly (one DVE op fewer) ---
                        fT_ps = ptr.tile([128, 512], bf16, tag="tr")
                        for p in range(4):
                            nc.tensor.transpose(
                                out=fT_ps[:, 128 * p:128 * (p + 1)],
                                in_=f_sb[:, 128 * p:128 * (p + 1)],
                                identity=c_id_bf[:],
                            )
                        fT = qsb.tile([128, 8, 65], bf16, tag="fT")
                        nc.scalar.activation(
                            out=fT[:, :, 64:65],
                            in_=ex_ps[:].rearrange("p (c o) -> p c o", o=1),
                            func=AF.Copy)
                        nc.vector.tensor_tensor(
                            out=fT[:, :, 0:64],
                            in0=fT_ps[:].rearrange("p (c d) -> p c d", d=64),
                            in1=ex_em[:].rearrange("p (c o) -> p c o", o=1)
                                .to_broadcast([128, 8, W]),
                            op=mybir.AluOpType.mult,
                        )

                        # --- segment-sum cell partials ---
                        for t8 in range(8):
                            t = qt0 + t8
                            g = got[t]
                            q = sot[t]
                            ct = Cq[q] // 128
                            j = (t - blk_first[q]) % ct
                            key = (g, q)
                            if j == 0:
                                G_cell[key] = pg.tile([65, W], f32, tag="G",
                                                      name=f"G{g}_{q}")
                            G_ps = G_cell[key]
                            nc.tensor.matmul(
                                G_ps[:], lhsT=fT[:, t8, :], rhs=oh2[:, t8, :],
                                start=(j == 0), stop=(j == ct - 1),
                                skip_group_check=True)
                            if j == ct - 1:
                                if q == first_q:
                                    nc.scalar.activation(out=G_all[:, g, :],
                                                         in_=G_ps[:],
                                                         func=AF.Copy)
                                else:
                                    nc.vector.tensor_tensor(
                                        out=G_all[:, g, :], in0=G_all[:, g, :],
                                        in1=G_ps[:], op=mybir.AluOpType.add)
                                del G_cell[key]
                                if q == last_q:
                                    group_post(g)
    nc.finalize()
    return nc


# ----------------------------------------------------------------------------
# public entry point
# ----------------------------------------------------------------------------

_CACHE = {}


def _get_nc(meta):
    key = (meta["T"], meta["NG"], meta["B_loc"], meta["n_cores"], meta["Cq"],
           meta["UR"], meta["IR"], meta["call_t"], meta["call_caps"],
           meta.get("ver"), meta.get("queues", 1))
    if key not in _CACHE:
        _CACHE[key] = build_nc_real(meta)
    return _CACHE[key]


def make_in_maps(per_core, shared, meta):
    in_maps = []
    for k in range(meta["n_cores"]):
        m = dict(shared)
        m.update(per_core[k])
        in_maps.append(m)
    return in_maps


def kernel(**inputs):
    per_core, shared, meta = host_prep(inputs, N_CORES)
    if QUEUES > 1:
        meta["queues"] = QUEUES
    nc = _get_nc(meta)
    in_maps = make_in_maps(per_core, shared, meta)
    res = run_bass_kernel_spmd(nc, in_maps, core_ids=list(range(N_CORES)))
    outs = [res.results[k]["out"] for k in range(N_CORES)]
    return np.concatenate(outs, axis=0).astype(np.float32)


# revision 29
# speedup vs baseline: 1.0071x; 1.0071x over previous
"""Trainium2 Bass kernel for nn_Aggre_user (GNN message-passing aggregation).

Reference computation (per batch node, over its variable-length edge list):
    f      = relu(ln1(cat(user_emb, rating_emb)))            per edge
    h      = relu(att2(relu(att1(cat(f, item_emb[node])))))  per edge
    a      = att3(h)                                         per edge logit
    mu     = segment_softmax(a)
    z      = relu(ln2(segment_sum(f * mu)))                  per node
    out    = relu(ln3(cat(item_emb[node], z)))               per node

Sharding: nodes (B=8192) split contiguously across 8 cores (1024 each);
segment_ids are sorted, so each node's edges land wholly on one core.
No collectives needed.

V2 design (vs. the earlier edge-major baseline):
  - The user table is folded through ln1 on the HOST: ctab[u] = bf16(W1u @ u)
    stored as 256B rows [c_u(64) | zeros(64)].  The per-edge gather runs the
    ANT dma_gather ucode in TRANSPOSE mode, so c_u arrives FEATURE-MAJOR:
    no ln1-u matmul, no PE transposes of u, no dtype cast.
  - Per-call num_idxs is capped at the 8-core max real-slot count of the
    window (128-aligned for transpose mode) so all-pad tails cost nothing.
    (Shipping pads as -1 for the ucode's per-core self-trim measured as a
    hard crash in transpose mode -- the rx spray path desyncs -- so pads
    are plain index 0 and their contributions die in the segment one-hot.)
  - Tile pairs [cA; cB] are assembled for the 128-row block-diagonal
    pipeline with 4 SBUF->SBUF DMA copies per quad (DMA engines are
    otherwise idle; the bottom halves need a partition shift that DVE
    lanes cannot do), then ONE full-width [128,512] DVE add folds them
    onto the rating PSUM. Half-partition [64,x] DVE ops run at half rate
    and also stall the gather ucode via the shared SBUF port -- avoid.
  - att1's per-node one-hot (node-major oh1) is built WITHOUT PE transposes:
    a 2-row broadcast matmul (lhsT = half-selector, rhs = host-shipped
    pairwise seg-local ids) replicates seg ids down the partitions, then one
    DVE is_equal against a per-partition iota gives oh1 directly.
  - Item embeddings: host packs FOUR items per 1024B row -> the item table
    has 25000 rows = ONE int16 window; four 256-idx transpose-mode calls
    (1024B elems generate 4 rx descriptors per index, so a single 1024-idx
    call would overflow the 32KB SWDGE descriptor ring) fetch all nodes'
    items in one pass; chunk masks select the right 64-row slot.
  - rating table folded into ln1 via host-built 5-hot and M_r = rating_table
    @ ln1_w[:,64:].T streamed as extra contraction rows (10-row lhsT).
  - segment softmax: no max-subtraction needed (logits O(1)); no explicit mu:
    z = (sum_e ex*f) / max(sum_e ex, 1e-9); both sums via one matmul with an
    ex-scaled one-hot and a ones column (per shard-cell partials, summed).
"""

import math
import numpy as np
import ml_dtypes

import concourse.bass as bass
import concourse.mybir as mybir
import concourse.tile as tile
from concourse import bacc
from concourse.bass_utils import run_bass_kernel_spmd

BF16 = ml_dtypes.bfloat16
AF = mybir.ActivationFunctionType
N_CORES = 8
W = 64        # nodes per segment group
SH = 25000    # ids per gather shard
NSH = 4       # shards
CALL_T = 4    # tiles per user-gather call
NEG_TRIM = False  # -1 pads crash the transpose gather ucode (tx/rx desync)
QUEUES = 1    # SWDGE queues; queue q runs on Q7 core pair (2q, 2q+1)
NEG_ALIGN = 128  # per-core trim alignment


def pack_item_table(t):
    """[N, 64] f32 -> quad-packed bf16 rows [i0|0|i1|0|i2|0|i3|0] (1024B).

    25000 rows fit ONE int16 window, so the whole item gather is a single
    transpose-mode pass; per-node chunk masks select the right 64-row slot.
    """
    t = np.asarray(t, np.float32)
    N = t.shape[0]
    R = (N + 3) // 4
    out = np.zeros((R, 512), np.float32)
    for k in range(4):
        rows = t[k::4]
        out[: len(rows), 128 * k: 128 * k + 64] = rows
    return out.astype(BF16), R


def wrap16(idx_i16):
    """flat int16 index list (len % 16 == 0) -> [128, len//16] wrapped+replicated."""
    a = idx_i16.reshape(-1, 16).T  # [16, S]
    return np.tile(a, (8, 1)).copy()


# ----------------------------------------------------------------------------
# host-side preprocessing: shard + pad + relayout (pure index manipulation)
# ----------------------------------------------------------------------------

def host_prep(inputs, n_cores=N_CORES, call_t=CALL_T):
    user_idx = np.asarray(inputs["user_idx"]).astype(np.int64)
    rating_idx = np.asarray(inputs["rating_idx"]).astype(np.int64)
    item_idx = np.asarray(inputs["item_idx"]).astype(np.int64)
    seg = np.asarray(inputs["segment_ids"]).astype(np.int64)

    B = item_idx.shape[0]
    B_loc = B // n_cores
    assert B_loc % 128 == 0
    NG = B_loc // W
    n_groups = B // W

    f32 = np.float32
    ln1_w = np.asarray(inputs["ln1_w"], f32)

    # host-folded user table: ctab[u] = [W1u @ u | 0] in bf16 (256B rows)
    utab_f32 = np.ascontiguousarray(np.asarray(inputs["user_table"], f32))
    c_all = utab_f32 @ ln1_w[:, :64].T            # [N_u, 64] f32
    N_u = c_all.shape[0]
    ctab = np.zeros((N_u, 128), np.float32)
    ctab[:, 0:64] = c_all
    ctab = ctab.astype(BF16)

    itab, n_irows = pack_item_table(inputs["item_table"])
    n_ush = (N_u + SH - 1) // SH
    u_shard = user_idx // SH        # shard per edge

    bounds = np.searchsorted(seg, np.arange(n_groups + 1) * W)

    # per-(group, shard) cell counts -> per-shard cell capacity C_q
    cellcnt = np.zeros((n_groups, NSH), np.int64)
    for g in range(n_groups):
        lo, hi = bounds[g], bounds[g + 1]
        for q in range(NSH):
            cellcnt[g, q] = int((u_shard[lo:hi] == q).sum())
    # tiles per shard block must divide into CALL_T-tile gather calls:
    # NG * C_q / 128 % CALL_T == 0  <=>  C_q % (128 * CALL_T / NG) == 0
    align = max(128, 128 * call_t // NG)
    Cq = [0 if q >= n_ush else
          int(math.ceil(max(1, int(cellcnt[:, q].max())) / align) * align)
          for q in range(NSH)]
    E_grp = sum(Cq)
    T = NG * E_grp // 128
    assert T % 8 == 0
    E_pad = NG * E_grp
    blk_tiles = [NG * c // 128 for c in Cq]   # tiles per shard block
    group_of_tile = []
    for q in range(NSH):
        for g in range(NG):
            group_of_tile += [g] * (Cq[q] // 128)
    shard_of_tile = []
    for q in range(NSH):
        shard_of_tile += [q] * blk_tiles[q]

    per_core = []
    blk0 = np.concatenate([[0], np.cumsum(blk_tiles)]) * 128  # slot offsets

    # Within a cell, slot order is free. Place each cell's pad slots as one
    # run ending at the last 512-boundary inside the cell: every gather-call
    # window the run touches has it as a suffix. Suffix pads are shipped as
    # index -1: the ucode trims trailing negatives PER CORE, so their
    # descriptor generation cost vanishes. The static per-call num_idxs cap
    # (8-core max, rounded to 128 for transpose mode) also skips their
    # index-load cost where all 8 cores are padded.
    win = 128 * call_t
    n_calls = T // call_t
    all_lastreal = []
    for k in range(n_cores):
        ugl = np.zeros(E_pad, np.int64)      # shard-local padded positions
        ridx = np.full(E_pad, -1, np.int64)
        sloc = np.full(E_pad, -1.0, np.float64)
        is_pad = np.zeros(E_pad, bool)
        for gl in range(NG):
            g = NG * k + gl
            lo, hi = bounds[g], bounds[g + 1]
            esl = np.arange(lo, hi)
            shards_here = u_shard[lo:hi]
            for q in range(NSH):
                if Cq[q] == 0:
                    assert not (shards_here == q).any()
                    continue
                mine = esl[shards_here == q]
                c = len(mine)
                s = int(blk0[q]) + gl * Cq[q]
                assert c <= Cq[q]
                p = Cq[q] - c
                bnd = ((s + Cq[q]) // win) * win
                if bnd - p < s or bnd <= s:
                    bnd = s + Cq[q]  # no usable boundary: pads at cell end
                pos = np.concatenate([np.arange(s, bnd - p),
                                      np.arange(bnd, s + Cq[q])])
                assert len(pos) == c
                ugl[pos] = user_idx[mine] - SH * q
                is_pad[bnd - p:bnd] = True
                ridx[pos] = rating_idx[mine]
                sloc[pos] = seg[mine] - W * g
        assert (ugl >= 0).all() and (ugl < SH).all()
        # mark window-suffix pads as -1 (ucode self-trim); others stay 0.
        # The trimmed per-core count must stay a multiple of 16: the
        # transpose rx path sprays full 16-lane groups unconditionally, so a
        # ragged trim desyncs tx/rx descriptor counts (hang). Trim only from
        # the 16-aligned boundary above the last real slot.
        lastreal = []
        for c2 in range(n_calls):
            wsl = slice(c2 * win, (c2 + 1) * win)
            nz = np.nonzero(~is_pad[wsl])[0]
            last = int(nz[-1]) + 1 if len(nz) else 0
            lastreal.append(last)
            if NEG_TRIM:
                last16 = -(-last // NEG_ALIGN) * NEG_ALIGN
                ugl[c2 * win + last16:(c2 + 1) * win] = -1
        all_lastreal.append(lastreal)
        uw = wrap16(ugl.astype(np.int16))    # [128, E_pad//16]
        segl = sloc.reshape(T, 128).T.astype(BF16).copy()
        # pairwise seg-local ids for the oh1 broadcast matmul:
        # col 128*p + e -> (row0: tile 2p, row1: tile 2p+1)
        sl2 = sloc.reshape(T, 128)
        seglAB = np.stack([sl2[0::2].reshape(-1), sl2[1::2].reshape(-1)])
        rt = ridx.reshape(T, 128)
        P = T // 2
        oh5 = np.zeros((10, P, 128), np.float32)
        for r in range(5):
            oh5[r] = (rt[0::2] == r)
            oh5[5 + r] = (rt[1::2] == r)
        oh5p = oh5.reshape(10, P * 128).astype(BF16)
        # item gather: quad-packed single window, node order
        nodes = slice(B_loc * k, B_loc * (k + 1))
        it = item_idx[nodes]
        iw = wrap16((it // 4).astype(np.int16))
        masks = []
        for kk in range(4):
            m = np.tile((it % 4 == kk).astype(np.float32), (128, 1))
            masks.append(m.astype(BF16))
        per_core.append(dict(
            uw=uw, segl=segl, seglAB=seglAB.astype(BF16), oh5p=oh5p,
            iw=iw, im=np.stack(masks, axis=1),  # [128, 4, B_loc]
        ))

    call_caps = [
        min(win, -(-max(all_lastreal[k][c] for k in range(n_cores)) // 128) * 128)
        for c in range(n_calls)
    ]

    # weights (tiny; fold rating table into ln1 on host)
    att1_w = np.asarray(inputs["att1_w"], f32)
    att2_w = np.asarray(inputs["att2_w"], f32)
    att3_w = np.asarray(inputs["att3_w"], f32)
    ln2_w = np.asarray(inputs["ln2_w"], f32)
    ln3_w = np.asarray(inputs["ln3_w"], f32)
    rating_table = np.asarray(inputs["rating_table"], f32)

    def bd(a):
        K, M = a.shape
        o = np.zeros((2 * K, 2 * M), f32)
        o[:K, :M] = a
        o[K:, M:] = a
        return o.astype(BF16)

    MrT = rating_table @ ln1_w[:, 64:].T
    w3 = att3_w[0]
    w3p = np.zeros((128, 2), f32)
    w3p[:64, 0] = w3
    w3p[64:, 1] = w3

    sel2 = np.zeros((2, 128), f32)
    sel2[0, 0:64] = 1.0
    sel2[1, 64:128] = 1.0
    iota64p = np.tile(np.arange(W, dtype=f32), 2)[:, None]  # [128,1] p%64

    shared = dict(
        bd_mr=bd(MrT),
        bd_a1f=bd(att1_w[:, :64].T),
        a1it=att1_w[:, 64:].T.astype(BF16),
        bd_a2=bd(att2_w.T), w3p=w3p.astype(BF16),
        w2t=ln2_w.T.astype(BF16),
        w3it=ln3_w[:, :64].T.astype(BF16), w3zt=ln3_w[:, 64:].T.astype(BF16),
        b1p=np.tile(np.asarray(inputs["ln1_b"], f32), 2)[:, None],
        ba1p=np.tile(np.asarray(inputs["att1_b"], f32), 2)[:, None],
        ba2p=np.tile(np.asarray(inputs["att2_b"], f32), 2)[:, None],
        b2=np.asarray(inputs["ln2_b"], f32)[:, None],
        b3f=np.asarray(inputs["ln3_b"], f32)[:, None],
        iota64=np.tile(np.arange(W, dtype=f32), (128, 1)).astype(BF16),
        id_bf=np.eye(128, dtype=f32).astype(BF16),
        id_f32=np.eye(128, dtype=f32),
        sel2=sel2.astype(BF16), iota64p=iota64p.astype(f32),
        ctab=ctab, itab=itab,
    )
    meta = dict(B=B, B_loc=B_loc, NG=NG, T=T, E_grp=E_grp, Cq=tuple(Cq),
                blk_tiles=tuple(blk_tiles), group_of_tile=tuple(group_of_tile),
                shard_of_tile=tuple(shard_of_tile),
                call_caps=tuple(call_caps),
                n_cores=n_cores, UR=ctab.shape[0], IR=itab.shape[0],
                n_ush=n_ush, call_t=call_t, ver=2)
    return per_core, shared, meta


# ----------------------------------------------------------------------------
# bass program builder
# ----------------------------------------------------------------------------

def build_nc_real(meta):
    NG, T = meta["NG"], meta["T"]
    B_loc = meta["B_loc"]
    Cq, blk_tiles = meta["Cq"], meta["blk_tiles"]
    got = meta["group_of_tile"]
    sot = meta["shard_of_tile"]
    CT = meta["call_t"]
    assert T % CT == 0
    n_calls = T // CT

    nq = meta.get("queues", 1)
    nc = bacc.Bacc("TRN2", target_bir_lowering=False, debug=False,
                   enable_asserts=False, num_devices=meta["n_cores"],
                   dynamic_dma_scratch_size=32768, num_swdge_queues=nq)
    f32, bf16 = mybir.dt.float32, mybir.dt.bfloat16
    i16, i32 = mybir.dt.int16, mybir.dt.int32

    def din(name, shape, dtype):
        return nc.dram_tensor(name, shape, dtype, kind="ExternalInput").ap()

    ctab = din("ctab", [meta["UR"], 128], bf16)
    itab = din("itab", [meta["IR"], 512], bf16)
    uw = din("uw", [128, T * 8], i16)
    segl = din("segl", [128, T], bf16)
    seglAB = din("seglAB", [2, T * 64], bf16)
    oh5p = din("oh5p", [10, 64 * T], bf16)
    iw = din("iw", [128, B_loc // 16], i16)
    im = din("im", [128, 4, B_loc], bf16)
    iota64 = din("iota64", [128, W], bf16)
    id_bf = din("id_bf", [128, 128], bf16)
    id_f32 = din("id_f32", [128, 128], f32)
    sel2 = din("sel2", [2, 128], bf16)
    iota64p = din("iota64p", [128, 1], f32)
    bd_mr = din("bd_mr", [10, 128], bf16)
    bd_a1f = din("bd_a1f", [128, 128], bf16)
    a1it = din("a1it", [64, 64], bf16)
    bd_a2 = din("bd_a2", [128, 128], bf16)
    w3p = din("w3p", [128, 2], bf16)
    w2t = din("w2t", [64, 64], bf16)
    w3it = din("w3it", [64, 64], bf16)
    w3zt = din("w3zt", [64, 64], bf16)
    b1p = din("b1p", [128, 1], f32)
    ba1p = din("ba1p", [128, 1], f32)
    ba2p = din("ba2p", [128, 1], f32)
    b2 = din("b2", [64, 1], f32)
    b3f = din("b3f", [64, 1], f32)
    out = nc.dram_tensor("out", [B_loc, 64], f32, kind="ExternalOutput").ap()

    with tile.TileContext(nc) as tc:
        with (
            tc.tile_pool(name="const", bufs=1) as cpool,
            tc.tile_pool(name="core", bufs=1) as corep,
            tc.tile_pool(name="ug", bufs=12) as ugp,
            tc.tile_pool(name="stk", bufs=3) as stkp,
            tc.tile_pool(name="sab", bufs=3) as sabp,
            tc.tile_pool(name="qsb", bufs=4) as qsb,
            tc.tile_pool(name="post", bufs=2) as postp,
            tc.tile_pool(name="pmm", bufs=3, space="PSUM") as pmm,
            tc.tile_pool(name="ptr", bufs=2, space="PSUM") as ptr,
            tc.tile_pool(name="pa", bufs=1, space="PSUM") as pa,
            tc.tile_pool(name="pg", bufs=1, space="PSUM") as pg,
            tc.tile_pool(name="pgp", bufs=1, space="PSUM") as pgp,
        ):
            def load(pool, ap, tag):
                t = pool.tile(list(ap.shape), ap.dtype, tag=tag, name=tag)
                nc.sync.dma_start(out=t[:], in_=ap)
                return t

            c_id_bf = load(cpool, id_bf, "id_bf")
            c_id_f32 = load(cpool, id_f32, "id_f32")
            c_iota = load(cpool, iota64, "iota")
            c_sel2 = load(cpool, sel2, "sel2")
            c_iota64p = load(cpool, iota64p, "iota64p")
            c_bd_mr = load(cpool, bd_mr, "bd_mr")
            c_bd_a1f = load(cpool, bd_a1f, "bd_a1f")
            c_a1it = load(cpool, a1it, "a1it")
            c_bd_a2 = load(cpool, bd_a2, "bd_a2")
            c_w3p = load(cpool, w3p, "w3p")
            c_w2t = load(cpool, w2t, "w2t")
            c_w3it = load(cpool, w3it, "w3it")
            c_w3zt = load(cpool, w3zt, "w3zt")
            c_b1p = load(cpool, b1p, "b1p")
            c_ba1p = load(cpool, ba1p, "ba1p")
            c_ba2p = load(cpool, ba2p, "ba2p")
            c_b2 = load(cpool, b2, "b2")
            c_b3f = load(cpool, b3f, "b3f")
            c_segl = load(corep, segl, "segl")
            c_uw = load(corep, uw, "uw")
            c_oh5p = load(corep, oh5p, "oh5p")
            c_iw = load(corep, iw, "iw")
            c_im = load(corep, im, "im")

            # Zero the user-gather ring once: slots past a call's cap are
            # never written by the gather, so their SBUF content must be
            # finite (contributions are killed by the segment one-hot, but
            # Inf/NaN garbage would poison 0*x products downstream).
            for _i in range(12):
                zt = ugp.tile([128, CT * 128], bf16, tag="u_g",
                              name=f"ug_init{_i}")
                nc.vector.memset(zt[:], 0.0)

            dbg = meta.get("dbg", "")

            for _rep in range(meta.get("repeat", 1)):
                # ---- item embeddings: one quad-packed transpose pass ----
                # 1024B rows -> 4 rx descriptors per index; chunk calls to
                # 256 idxs so each call's descriptors fit the 32KB SWDGE ring.
                # Layout [128, call, chunk, idx]: each call's region is
                # contiguous; chunk k is read back with a strided view.
                ICH = 256
                NIC = B_loc // ICH
                gq = corep.tile([128, NIC, 4, ICH], bf16, tag="itg", name="itg")
                if "no_item" in dbg:
                    nc.vector.memset(gq[:], 0.0)
                else:
                    for c in range(NIC):
                        nc.gpsimd.dma_gather(
                            out_ap=gq[:, c, :, :],
                            in_ap=itab[:, :],
                            idxs_ap=c_iw[:, c * ICH // 16:(c + 1) * ICH // 16],
                            num_idxs=ICH, num_idxs_reg=ICH,
                            elem_size=512, transpose=True,
                            queue_num=c % nq)

                def gq_chunk(k):
                    return gq[:, :, k, :]                # [128, NIC, ICH]

                def imv(k):
                    return c_im[:, k, :].rearrange("p (c n) -> p c n", c=NIC)

                s0 = corep.tile([128, B_loc], bf16, tag="s0", name="s0")
                s1 = corep.tile([128, B_loc], bf16, tag="s1", name="s1")
                sv = [s.rearrange("p (c n) -> p c n", c=NIC)
                      for s in (s0[:], s1[:])]
                nc.vector.tensor_tensor(out=sv[0], in0=gq_chunk(0),
                                        in1=imv(0), op=mybir.AluOpType.mult)
                nc.vector.tensor_tensor(out=sv[1], in0=gq_chunk(1),
                                        in1=imv(1), op=mybir.AluOpType.mult)
                s2 = corep.tile([128, B_loc], bf16, tag="s2", name="s2")
                s3 = corep.tile([128, B_loc], bf16, tag="s3", name="s3")
                sv2 = [s.rearrange("p (c n) -> p c n", c=NIC)
                       for s in (s2[:], s3[:])]
                nc.vector.tensor_tensor(out=sv2[0], in0=gq_chunk(2),
                                        in1=imv(2), op=mybir.AluOpType.mult)
                nc.vector.tensor_tensor(out=sv2[1], in0=gq_chunk(3),
                                        in1=imv(3), op=mybir.AluOpType.mult)
                nc.vector.tensor_tensor(out=s0[:], in0=s0[:], in1=s1[:],
                                        op=mybir.AluOpType.add)
                nc.vector.tensor_tensor(out=s2[:], in0=s2[:], in1=s3[:],
                                        op=mybir.AluOpType.add)
                itemT = corep.tile([128, B_loc], bf16, tag="itemT", name="itemT")
                nc.vector.tensor_tensor(out=itemT[:], in0=s0[:], in1=s2[:],
                                        op=mybir.AluOpType.add)

                def item_fm_slice(g):
                    return itemT[0:64, W * g:W * g + W]

                # ---- per-group c1 block-diag lhsT ----
                bd_c1 = corep.tile([128, NG, 128], bf16, tag="bd_c1", name="bd_c1")
                nc.gpsimd.memset(bd_c1[:], 0)
                for g in range(NG):
                    src = item_fm_slice(g)
                    ps = pgp.tile([128, 128], f32, tag="gp", name=f"c1ps{g}")
                    nc.tensor.matmul(ps[0:64, 0:64], lhsT=src, rhs=c_a1it[:],
                                     start=True, stop=True, skip_group_check=True)
                    nc.tensor.matmul(ps[64:128, 64:128], lhsT=src, rhs=c_a1it[:],
                                     start=True, stop=True, skip_group_check=True)
                    nc.vector.tensor_copy(out=bd_c1[0:64, g, 0:64],
                                          in_=ps[0:64, 0:64])
                    nc.vector.tensor_copy(out=bd_c1[64:128, g, 64:128],
                                          in_=ps[64:128, 64:128])

                # per-group accumulated G (f32, SBUF)
                G_all = corep.tile([65, NG, W], f32, tag="G_all", name="G_all")

                def group_post(g):
                    G_sb = G_all[:, g, :]
                    Gt = pgp.tile([64, 65], f32, tag="gp", name=f"Gt{g}")
                    nc.tensor.transpose(out=Gt[:], in_=G_sb,
                                        identity=c_id_f32[0:65, 0:65])
                    den = postp.tile([64, 1], f32, tag="den", name=f"den{g}")
                    nc.vector.tensor_scalar_max(out=den[:], in0=Gt[:, 64:65],
                                                scalar1=1e-9)
                    rec = postp.tile([64, 1], f32, tag="rec", name=f"rec{g}")
                    nc.vector.reciprocal(out=rec[:], in_=den[:])
                    z_nm = postp.tile([64, W], bf16, tag="z_nm", name=f"znm{g}")
                    nc.vector.tensor_scalar_mul(out=z_nm[:], in0=Gt[:, 0:64],
                                                scalar1=rec[:, 0:1])
                    zf_ps = pgp.tile([64, 64], bf16, tag="gp", name=f"zf{g}")
                    nc.tensor.transpose(out=zf_ps[:], in_=z_nm[:],
                                        identity=c_id_bf[0:64, 0:64])
                    z_fm = postp.tile([64, 64], bf16, tag="z_fm", name=f"zfm{g}")
                    nc.vector.tensor_copy(out=z_fm[:], in_=zf_ps[:])
                    z2_ps = pgp.tile([64, 64], f32, tag="gp", name=f"z2ps{g}")
                    nc.tensor.matmul(z2_ps[:], lhsT=c_w2t[:], rhs=z_fm[:],
                                     start=True, stop=True, skip_group_check=True)
                    z2 = postp.tile([64, 64], bf16, tag="z2", name=f"z2{g}")
                    nc.scalar.activation(out=z2[:], in_=z2_ps[:], func=AF.Relu,
                                         bias=c_b2[:])
                    o_ps = pgp.tile([64, 64], f32, tag="gp", name=f"ops{g}")
                    nc.tensor.matmul(o_ps[:], lhsT=c_w3it[:], rhs=item_fm_slice(g),
                                     start=True, stop=False, skip_group_check=True)
                    nc.tensor.matmul(o_ps[:], lhsT=c_w3zt[:], rhs=z2[:],
                                     start=False, stop=True, skip_group_check=True)
                    o_fm = postp.tile([64, 64], f32, tag="o_fm", name=f"ofm{g}")
                    nc.scalar.activation(out=o_fm[:], in_=o_ps[:], func=AF.Relu,
                                         bias=c_b3f[:])
                    otr = pgp.tile([64, 64], f32, tag="gp", name=f"otr{g}")
                    nc.tensor.transpose(out=otr[:], in_=o_fm[:],
                                        identity=c_id_f32[0:64, 0:64])
                    o_sb = postp.tile([64, 64], f32, tag="o_sb", name=f"osb{g}")
                    nc.vector.tensor_copy(out=o_sb[:], in_=otr[:])
                    nc.sync.dma_start(out=out[W * g:W * g + W, :], in_=o_sb[:])

                # ---- main edge pipeline over gather calls / quads ----
                blk_first = [sum(blk_tiles[:q]) for q in range(NSH)]
                last_q = max(q for q in range(NSH) if blk_tiles[q] > 0)
                first_q = min(q for q in range(NSH) if blk_tiles[q] > 0)
                G_cell = {}

                u_call = {}
                caps = meta["call_caps"]

                def ensure_call(call):
                    if call in u_call:
                        return
                    t0 = call * CT
                    q_shard = sot[t0]
                    base = SH * q_shard
                    rows = min(SH, meta["UR"] - base)
                    cap = caps[call]
                    u_g = ugp.tile([128, CT * 128], bf16, tag="u_g",
                                   name=f"ug{call}")
                    if "no_ug" in dbg:
                        nc.vector.memset(u_g[:], 0.0)
                    elif cap > 0:
                        nc.gpsimd.dma_gather(
                            out_ap=u_g[:].rearrange("p (o n) -> p o n",
                                                    o=1)[:, :, 0:cap],
                            in_ap=ctab[base:base + rows, :],
                            idxs_ap=c_uw[:, t0 * 8:t0 * 8 + cap // 16],
                            num_idxs=cap, num_idxs_reg=cap,
                            elem_size=128, transpose=True,
                            queue_num=call % nq)
                    u_call[call] = u_g

                for quad in range(T // 8):
                    if True:
                        qt0 = quad * 8
                        cA = qt0 // CT
                        cB = cA + 1
                        for c in (cA, cB):
                            ensure_call(c)
                        if meta.get("only_gather"):
                            continue

                        # --- bottom halves of the tile pairs need a
                        # partition shift: SBUF->SBUF DMA from gather ring
                        # rows 0:64 (odd tiles) into stk rows 64:128 ---
                        stk = stkp.tile([128, 512], bf16, tag="stk")
                        if "no_stk" in dbg:
                            nc.vector.memset(stk[:], 0.0)
                        else:
                            for ci, cc in enumerate((cA, cB)):
                                nc.sync.dma_start(
                                    out=stk[0:64, 256 * ci:256 * ci + 256]
                                    .rearrange("p (b n) -> p b n", b=2),
                                    in_=u_call[cc][0:64, :]
                                    .rearrange("p (b n) -> p b n", b=4)[:, 0::2, :])
                                nc.sync.dma_start(
                                    out=stk[64:128, 256 * ci:256 * ci + 256]
                                    .rearrange("p (b n) -> p b n", b=2),
                                    in_=u_call[cc][0:64, :]
                                    .rearrange("p (b n) -> p b n", b=4)[:, 1::2, :])

                        # --- ln1: rating part on PE, user part added in ---
                        f_ps = pmm.tile([128, 512], f32, tag="mm")
                        nc.tensor.matmul(
                            f_ps[:], lhsT=c_bd_mr[:],
                            rhs=c_oh5p[:, 64 * qt0:64 * qt0 + 512],
                            start=True, stop=True, skip_group_check=True)
                        f_pre = qsb.tile([128, 512], bf16, tag="fpre")
                        nc.vector.tensor_tensor(
                            out=f_pre[:],
                            in0=f_ps[:],
                            in1=stk[:],
                            op=mybir.AluOpType.add)
                        f_sb = qsb.tile([128, 512], bf16, tag="f")
                        nc.scalar.activation(out=f_sb[:], in_=f_pre[:],
                                             func=AF.Relu, bias=c_b1p[:])

                        # --- segment one-hots ---
                        oh2 = qsb.tile([128, 8, W], bf16, tag="oh2")
                        nc.vector.tensor_tensor(
                            out=oh2[:],
                            in0=c_segl[:, qt0:qt0 + 8].to_broadcast([128, 8, W]),
                            in1=c_iota[:].rearrange("p (o n) -> p o n", o=1)
                                .to_broadcast([128, 8, W]),
                            op=mybir.AluOpType.is_equal,
                        )
                        # oh1 (node-major) via 2-row broadcast matmul +
                        # per-partition iota compare (no PE transposes)
                        sAB = sabp.tile([2, 512], bf16, tag="sab")
                        nc.sync.dma_start(
                            out=sAB[:], in_=seglAB[:, 64 * qt0:64 * qt0 + 512])
                        bc_ps = pmm.tile([128, 512], f32, tag="mm")
                        nc.tensor.matmul(
                            bc_ps[:], lhsT=c_sel2[:],
                            rhs=sAB[:],
                            start=True, stop=True, skip_group_check=True)
                        oh1 = qsb.tile([128, 512], bf16, tag="oh1")
                        nc.vector.tensor_scalar(
                            out=oh1[:], in0=bc_ps[:], scalar1=c_iota64p[:],
                            scalar2=None, op0=mybir.AluOpType.is_equal)

                        # --- att1 (f-part batched; c1-part per group run) ---
                        h1_ps = pmm.tile([128, 512], f32, tag="mm")
                        nc.tensor.matmul(h1_ps[:], lhsT=c_bd_a1f[:],
                                         rhs=f_sb[:], start=True,
                                         stop=False, skip_group_check=True)
                        p = 0
                        while p < 4:
                            g = got[qt0 + 2 * p]
                            p2 = p
                            while p2 < 4 and got[qt0 + 2 * p2] == g:
                                p2 += 1
                            cs = slice(128 * p, 128 * p2)
                            nc.tensor.matmul(h1_ps[:, cs], lhsT=bd_c1[:, g, :],
                                             rhs=oh1[:, cs], start=False,
                                             stop=True, skip_group_check=True)
                            p = p2
                        h1 = qsb.tile([128, 512], bf16, tag="h1")
                        nc.scalar.activation(out=h1[:], in_=h1_ps[:], func=AF.Relu,
                                             bias=c_ba1p[:])

                        # --- att2 ---
                        h2_ps = pmm.tile([128, 512], f32, tag="mm")
                        nc.tensor.matmul(h2_ps[:], lhsT=c_bd_a2[:],
                                         rhs=h1[:], start=True, stop=True,
                                         skip_group_check=True)
                        h2 = qsb.tile([128, 512], bf16, tag="h2")
                        nc.scalar.activation(out=h2[:], in_=h2_ps[:],
                                             func=AF.Relu, bias=c_ba2p[:])

                        # --- att3 logits + exp ---
                        a_ps = pa.tile([2, 512], f32, tag="a")
                        nc.tensor.matmul(a_ps[:], lhsT=c_w3p[:], rhs=h2[:],
                                         start=True, stop=True, skip_group_check=True)
                        ex = qsb.tile([2, 512], bf16, tag="ex")
                        nc.scalar.activation(out=ex[:], in_=a_ps[:], func=AF.Exp)

                        ex_ps = ptr.tile([128, 8], bf16, tag="tr")
                        for p in range(4):
                            nc.tensor.transpose(
                                out=ex_ps[:, 2 * p:2 * p + 2],
                                in_=ex[:, 128 * p:128 * (p + 1)],
                                identity=c_id_bf[0:2, 0:2],
                            )
                        ex_em = qsb.tile([128, 8], bf16, tag="ex_em")
                        nc.vector.tensor_copy(out=ex_em[:], in_=ex_ps[:])

                        # --- f to edge-major, ex folded in: fT = [f*ex; ex]
                        # so G's rhs is oh2 directly (one DVE op fewer) ---
                        fT_ps = ptr.tile([128, 512], bf16, tag="tr")
                        for p in range(4):
                            nc.tensor.transpose(
                                out=fT_ps[:, 128 * p:128 * (p + 1)],
                                in_=f_sb[:, 128 * p:128 * (p + 1)],
                                identity=c_id_bf[:],
                            )
                        fT = qsb.tile([128, 8, 65], bf16, tag="fT")
                        nc.vector.tensor_copy(
                            out=fT[:, :, 64:65],
                            in_=ex_em[:].rearrange("p (c o) -> p c o", o=1))
                        nc.vector.tensor_tensor(
                            out=fT[:, :, 0:64],
                            in0=fT_ps[:].rearrange("p (c d) -> p c d", d=64),
                            in1=ex_em[:].rearrange("p (c o) -> p c o", o=1)
                                .to_broadcast([128, 8, W]),
                            op=mybir.AluOpType.mult,
                        )

                        # --- segment-sum cell partials ---
                        for t8 in range(8):
                            t = qt0 + t8
                            g = got[t]
                            q = sot[t]
                            ct = Cq[q] // 128
                            j = (t - blk_first[q]) % ct
                            key = (g, q)
                            if j == 0:
                                G_cell[key] = pg.tile([65, W], f32, tag="G",
                                                      name=f"G{g}_{q}")
                            G_ps = G_cell[key]
                            nc.tensor.matmul(
                                G_ps[:], lhsT=fT[:, t8, :], rhs=oh2[:, t8, :],
                                start=(j == 0), stop=(j == ct - 1),
                                skip_group_check=True)
                            if j == ct - 1:
                                if q == first_q:
                                    nc.vector.tensor_copy(out=G_all[:, g, :],
                                                          in_=G_ps[:])
                                else:
                                    nc.vector.tensor_tensor(
                                        out=G_all[:, g, :], in0=G_all[:, g, :],
                                        in1=G_ps[:], op=mybir.AluOpType.add)
                                del G_cell[key]
                                if q == last_q:
                                    group_post(g)
    nc.finalize()
    return nc


# ----------------------------------------------------------------------------
# public entry point
# ----------------------------------------------------------------------------

_CACHE = {}


def _get_nc(meta):
    key = (meta["T"], meta["NG"], meta["B_loc"], meta["n_cores"], meta["Cq"],
           meta["UR"], meta["IR"], meta["call_t"], meta["call_caps"],
           meta.get("ver"), meta.get("queues", 1))
    if key not in _CACHE:
        _CACHE[key] = build_nc_real(meta)
    return _CACHE[key]


def make_in_maps(per_core, shared, meta):
    in_maps = []
    for k in range(meta["n_cores"]):
        m = dict(shared)
        m.update(per_core[k])
        in_maps.append(m)
    return in_maps


def kernel(**inputs):
    per_core, shared, meta = host_prep(inputs, N_CORES)
    if QUEUES > 1:
        meta["queues"] = QUEUES
    nc = _get_nc(meta)
    in_maps = make_in_maps(per_core, shared, meta)
    res = run_bass_kernel_spmd(nc, in_maps, core_ids=list(range(N_CORES)))
    outs = [res.results[k]["out"] for k in range(N_CORES)]
    return np.concatenate(outs, axis=0).astype(np.float32)
